# revision 1
# baseline (speedup 1.0000x reference)
"""Trainium2 Bass kernel, v2: tail-folded layout for full DVE lane use.

Same math as kernel.py. Difference: the y tail rows (y=128..191, 64
rows) of pairs of z-planes are folded into one 128-partition tile —
partitions 0:64 hold the first half of the chunk's planes, partitions
64:128 the second half (with a 2-plane overlap of the input slots so
z-derivative shifts stay uniform in the free dim). Every DVE op then
runs with all 128 lanes. PE matmuls on upper-half K-tiles use weight
copies stored at partition base 64 (legal 32-aligned base).
"""

import sys

sys.path.insert(0, "/opt/trn_rl_repo")

import numpy as np

N = 192
NCORES = 8

MU_REF = 1.8e-5
T_REF = 300.0
PR = 0.72
CP = 1005.0
C1 = N / 2.0
CLN = float(np.log(np.float32(MU_REF) * C1 * C1))
CPR = float(np.float32(CP / PR))
TWO3 = float(np.float32(2.0 / 3.0))


def build_program(nz=24, za=4, zb=4, num_devices=NCORES):
    import concourse.bacc as bacc
    import concourse.mybir as mybir
    from concourse.tile import TileContext

    f32 = mybir.dt.float32
    nt = nz + 2
    nc = bacc.Bacc("TRN2", target_bir_lowering=False, debug=False,
                   num_devices=num_devices)

    u_d = nc.dram_tensor("u", [3, nz + 4, N, N], f32, kind="ExternalInput")
    t_d = nc.dram_tensor("T", [nz + 4, N, N], f32, kind="ExternalInput")
    dyt_d = nc.dram_tensor("dyt", [N, N], f32, kind="ExternalInput")
    out_d = nc.dram_tensor("out", [4, nz, N, N], f32, kind="ExternalOutput")

    with TileContext(nc) as tc:
        with (
            tc.tile_pool(name="wpool", bufs=1) as wpool,
            tc.tile_pool(name="dram", bufs=1, space="DRAM") as dpool,
            tc.tile_pool(name="psum", bufs=4, space="PSUM") as pspool,
        ):
            clnt = wpool.tile([128, 1], f32, tag="cln")
            nc.vector.memset(clnt[:, :], CLN)

            # Dy^T blocks; (kt) 0=main K rows y0:128, 1=tail K rows y128:192
            # wd[kt][mt] at base 0; wd64[kt=1][mt] at partition base 64.
            dT = dyt_d.ap()
            wd = {}
            wd64 = {}
            for kt, (k0, nk) in enumerate([(0, 128), (128, 64)]):
                for mt, (m0, nm) in enumerate([(0, 128), (128, 64)]):
                    w = wpool.tile([nk, nm], f32, tag=f"wd{kt}{mt}")
                    nc.sync.dma_start(out=w[:, :],
                                      in_=dT[k0:k0 + nk, m0:m0 + nm])
                    wd[(kt, mt)] = w

            bz = dpool.tile([4, nt, N, N], f32, tag="bz")
            by = dpool.tile([4, nt, N, N], f32, tag="by")
            bx = dpool.tile([4, nt, N, N], f32, tag="bx")

            pe_stg_pool = [None]

            def pe_dy(scr, main_ctr, tail_feed, dy0, dy1, npl):
                """y-derivs of 4 fields x npl planes.

                main_ctr: [128, 4, npl, N]; tail_feed: [64, 4, npl, N]
                (base-0 copy of tail rows). dy0: [128,4,npl,N]; dy1:
                folded [128, 4, npl/2, N] (parts 0:64 first half planes).
                Upper-half tail drains stage through base-0 then DMA-hop.
                """
                h = npl // 2 if npl > 1 else 1
                for p in range(npl):
                    lo = p < h
                    for f0 in (0, 2):
                        nw = 2 * N
                        ps = pspool.tile([128, nw], f32, tag="ps0")
                        nc.tensor.matmul(ps[:, :], wd[(0, 0)][:, :],
                                         main_ctr[:, f0:f0 + 2, p, :],
                                         start=True, stop=False)
                        nc.tensor.matmul(ps[:, :], wd[(1, 0)][:, :],
                                         tail_feed[:, f0:f0 + 2, p, :],
                                         start=False, stop=True)
                        nc.scalar.copy(
                            dy0[:, f0:f0 + 2, p, :],
                            ps[:, :].rearrange("p (f x) -> p f x", f=2))
                        pt = pspool.tile([64, nw], f32, tag="ps1")
                        nc.tensor.matmul(pt[:, :], wd[(0, 1)][:, :],
                                         main_ctr[:, f0:f0 + 2, p, :],
                                         start=True, stop=False)
                        nc.tensor.matmul(pt[:, :], wd[(1, 1)][:, :],
                                         tail_feed[:, f0:f0 + 2, p, :],
                                         start=False, stop=True)
                        ptv = pt[:, :].rearrange("p (f x) -> p f x", f=2)
                        if lo:
                            nc.scalar.copy(dy1[0:64, f0:f0 + 2, p, :], ptv)
                        else:
                            stg = pe_stg_pool[0].tile([64, nw], f32, tag="stg")
                            sgv = stg.rearrange("p (f x) -> p f x", f=2)
                            nc.scalar.copy(sgv[:, :, :], ptv)
                            nc.sync.dma_start(
                                out=dy1[64:128, f0:f0 + 2, p - h, :],
                                in_=sgv[:, :, :])

            def compute_block(mybir, scr, v_ctr, dz, dx, dy, zc, suf):
                """Shared tau/e computation on [128, 4, zc, N] views.
                Returns (rv, ev) with 3-field row blocks / e columns."""
                p = 128
                lt = scr.tile([p, zc * N], f32, tag="lt")
                ltv = lt.rearrange("p (z x) -> p z x", z=zc)
                nc.scalar.activation(ltv[:, :, :], v_ctr[:, 3, :, :],
                                     mybir.ActivationFunctionType.Ln)
                mu = scr.tile([p, zc * N], f32, tag="mu")
                muv = mu.rearrange("p (z x) -> p z x", z=zc)
                nc.scalar.activation(muv[:, :, :], ltv[:, :, :],
                                     mybir.ActivationFunctionType.Exp,
                                     bias=clnt[0:p, :], scale=0.7)
                mut = scr.tile([p, zc * N], f32, tag="mut")
                mutv = mut.rearrange("p (z x) -> p z x", z=zc)
                nc.scalar.mul(mut[:, :], mu[:, :], CPR)

                dv = scr.tile([p, zc * N], f32, tag="dv")
                dvv = dv.rearrange("p (z x) -> p z x", z=zc)
                nc.vector.tensor_add(dvv[:, :, :], dz[:, 0, :, :],
                                     dx[:, 2, :, :])
                dv2 = scr.tile([p, zc * N], f32, tag="lt")
                dvv2 = dv2.rearrange("p (z x) -> p z x", z=zc)
                nc.vector.tensor_add(dvv2[:, :, :], dvv[:, :, :],
                                     dy[:, 1, :, :])
                q = scr.tile([p, zc * N], f32, tag="dv")
                qv = q.rearrange("p (z x) -> p z x", z=zc)
                nc.scalar.mul(q[:, :], dv2[:, :], TWO3)

                egt = scr.tile([p, 3 * zc * N], f32, tag="eg")
                eg = egt.rearrange("p (f z x) -> p f z x", f=3, z=zc)
                nc.vector.tensor_mul(eg[:, 0, :, :], mutv[:, :, :],
                                     dz[:, 3, :, :])
                nc.vector.tensor_mul(eg[:, 1, :, :], mutv[:, :, :],
                                     dy[:, 3, :, :])
                nc.vector.tensor_mul(eg[:, 2, :, :], mutv[:, :, :],
                                     dx[:, 3, :, :])

                rv = []
                for i in range(3):
                    rt = scr.tile([p, 3 * zc * N], f32, tag=f"r{i}")
                    rv.append(rt.rearrange("p (f z x) -> p f z x",
                                           f=3, z=zc))
                hb = scr.tile([p, 3 * zc * N], f32, tag="hb")
                hv = hb.rearrange("p (f z x) -> p f z x", f=3, z=zc)
                stt = nc.vector.scalar_tensor_tensor
                mub3 = muv.unsqueeze(1).broadcast_to((p, 3, zc, N))
                mub2 = muv.unsqueeze(1).broadcast_to((p, 2, zc, N))
                stt(hv[:, 0, :, :], dz[:, 0, :, :], 2.0, qv[:, :, :],
                    mybir.AluOpType.mult, mybir.AluOpType.subtract)
                nc.vector.tensor_add(hv[:, 1, :, :], dy[:, 0, :, :],
                                     dz[:, 1, :, :])
                nc.vector.tensor_add(hv[:, 2, :, :], dx[:, 0, :, :],
                                     dz[:, 2, :, :])
                nc.vector.tensor_mul(rv[0][:, :, :, :], hv[:, :, :, :], mub3)
                stt(hv[:, 1, :, :], dy[:, 1, :, :], 2.0, qv[:, :, :],
                    mybir.AluOpType.mult, mybir.AluOpType.subtract)
                nc.vector.tensor_add(hv[:, 2, :, :], dx[:, 1, :, :],
                                     dy[:, 2, :, :])
                nc.vector.tensor_mul(rv[1][:, 1:3, :, :],
                                     hv[:, 1:3, :, :], mub2)
                nc.sync.dma_start(out=rv[1][:, 0, :, :],
                                  in_=rv[0][:, 1, :, :])
                stt(hv[:, 2, :, :], dx[:, 2, :, :], 2.0, qv[:, :, :],
                    mybir.AluOpType.mult, mybir.AluOpType.subtract)
                nc.vector.tensor_mul(rv[2][:, 2, :, :], hv[:, 2, :, :],
                                     muv[:, :, :])
                nc.sync.dma_start(out=rv[2][:, 0, :, :],
                                  in_=rv[0][:, 2, :, :])
                nc.sync.dma_start(out=rv[2][:, 1, :, :],
                                  in_=rv[1][:, 2, :, :])

                pb = scr.tile([p, 3 * zc * N], f32, tag="dx")
                pbv = pb.rearrange("p (f z x) -> p f z x", f=3, z=zc)
                accs = [eg]
                for i in range(3):
                    ui = v_ctr[:, i:i + 1, :, :].broadcast_to((p, 3, zc, N))
                    nc.vector.tensor_mul(pbv[:, :, :, :],
                                         rv[i][:, :, :, :], ui)
                    na = scr.tile([p, 3 * zc * N], f32,
                                  tag=("dz" if i % 2 == 0 else "hb"))
                    nav = na.rearrange("p (f z x) -> p f z x", f=3, z=zc)
                    nc.vector.tensor_add(nav[:, :, :, :],
                                         accs[-1][:, :, :, :],
                                         pbv[:, :, :, :])
                    accs.append(nav)
                return rv, accs[-1]

            import concourse.mybir as mybir_mod

            # =============== PASS A ===============
            pass_a = tc.tile_pool(name="a_io", bufs=2)
            iopool = pass_a.__enter__()
            pe_stg_pool[0] = iopool
            scr_cm = tc.tile_pool(name="a_scr", bufs=1)
            scr = scr_cm.__enter__()
            t = -1
            while t < nz + 1:
                cza = min(za, nz + 1 - t)
                assert cza % 2 == 0, "za and nt must keep chunks even"
                hc = cza // 2
                ip0 = t + 1

                # main input [128, 4, cza+2, N]
                ti0 = iopool.tile([128, 4 * (cza + 2) * N], f32, tag="in0")
                v0 = ti0.rearrange("p (f z x) -> p f z x", f=4, z=cza + 2)
                for fi in range(3):
                    nc.sync.dma_start(
                        out=v0[:, fi, :, :],
                        in_=u_d.ap()[fi, ip0:ip0 + cza + 2, 0:128, :]
                        .transpose([1, 0, 2]))
                nc.sync.dma_start(
                    out=v0[:, 3, :, :],
                    in_=t_d.ap()[ip0:ip0 + cza + 2, 0:128, :]
                    .transpose([1, 0, 2]))
                # folded tail input [128, 4, hc+2, N]
                ti1 = iopool.tile([128, 4 * (hc + 2) * N], f32, tag="in1")
                v1 = ti1.rearrange("p (f z x) -> p f z x", f=4, z=hc + 2)
                for half, pofs in ((0, 0), (1, 64)):
                    p0 = ip0 + half * hc
                    for fi in range(3):
                        nc.sync.dma_start(
                            out=v1[pofs:pofs + 64, fi, :, :],
                            in_=u_d.ap()[fi, p0:p0 + hc + 2, 128:192, :]
                            .transpose([1, 0, 2]))
                    nc.sync.dma_start(
                        out=v1[pofs:pofs + 64, 3, :, :],
                        in_=t_d.ap()[p0:p0 + hc + 2, 128:192, :]
                        .transpose([1, 0, 2]))

                # base-0 tail feed for PE (duplicate load of center rows)
                tft = iopool.tile([64, 4 * cza * N], f32, tag="tf")
                tf = tft.rearrange("p (f z x) -> p f z x", f=4, z=cza)
                for fi in range(3):
                    nc.sync.dma_start(
                        out=tf[:, fi, :, :],
                        in_=u_d.ap()[fi, ip0 + 1:ip0 + 1 + cza, 128:192, :]
                        .transpose([1, 0, 2]))
                nc.sync.dma_start(
                    out=tf[:, 3, :, :],
                    in_=t_d.ap()[ip0 + 1:ip0 + 1 + cza, 128:192, :]
                    .transpose([1, 0, 2]))

                # PE y-derivs
                d0t = iopool.tile([128, 4 * cza * N], f32, tag="dy0")
                dy0 = d0t.rearrange("p (f z x) -> p f z x", f=4, z=cza)
                d1t = iopool.tile([128, 4 * hc * N], f32, tag="dy1")
                dy1 = d1t.rearrange("p (f z x) -> p f z x", f=4, z=hc)
                pe_dy(scr, v0[:, :, 1:1 + cza, :], tf, dy0, dy1, cza)

                for (vv, dyv, zc, suf) in ((v0, dy0, cza, "A"),
                                           (v1, dy1, hc, "B")):
                    ctr = vv[:, :, 1:1 + zc, :]
                    dzt = scr.tile([128, 4 * zc * N], f32, tag="dz")
                    dz = dzt.rearrange("p (f z x) -> p f z x", f=4, z=zc)
                    nc.vector.tensor_sub(dz[:, :, :, :],
                                         vv[:, :, 2:2 + zc, :],
                                         vv[:, :, 0:zc, :])
                    dxt = scr.tile([128, 4 * zc * N], f32, tag="dx")
                    dx = dxt.rearrange("p (f z x) -> p f z x", f=4, z=zc)
                    nc.vector.tensor_sub(dx[:, :, :, 1:191],
                                         ctr[:, :, :, 2:192],
                                         ctr[:, :, :, 0:190])
                    nc.vector.tensor_sub(dx[:, :, :, 0:192:191],
                                         ctr[:, :, :, 1::-1],
                                         ctr[:, :, :, 191:189:-1])

                    rv, ev = compute_block(mybir_mod, scr, ctr, dz, dx,
                                           dyv, zc, suf)

                    tt0 = t + 1
                    for buf, row in ((bz, 0), (by, 1), (bx, 2)):
                        if suf == "A":
                            for fi in range(3):
                                nc.sync.dma_start(
                                    out=buf[fi, tt0:tt0 + zc, 0:128, :]
                                    .transpose([1, 0, 2]),
                                    in_=rv[row][:, fi, :, :])
                            nc.sync.dma_start(
                                out=buf[3, tt0:tt0 + zc, 0:128, :]
                                .transpose([1, 0, 2]),
                                in_=ev[:, row, :, :])
                        else:
                            for half, pofs in ((0, 0), (1, 64)):
                                s0 = tt0 + half * hc
                                for fi in range(3):
                                    nc.sync.dma_start(
                                        out=buf[fi, s0:s0 + hc, 128:192, :]
                                        .transpose([1, 0, 2]),
                                        in_=rv[row][pofs:pofs + 64, fi, :, :])
                                nc.sync.dma_start(
                                    out=buf[3, s0:s0 + hc, 128:192, :]
                                    .transpose([1, 0, 2]),
                                    in_=ev[pofs:pofs + 64, row, :, :])
                t += cza

            scr_cm.__exit__(None, None, None)
            pass_a.__exit__(None, None, None)

            # =============== PASS B ===============
            pass_b = tc.tile_pool(name="b_io", bufs=2)
            iopool = pass_b.__enter__()
            pe_stg_pool[0] = iopool
            scrb_cm = tc.tile_pool(name="b_scr", bufs=1)
            scr = scrb_cm.__enter__()
            z = 0
            while z < nz:
                czb = min(zb, nz - z)
                assert czb % 2 == 0
                hb = czb // 2
                tt0 = z + 1

                lz0t = iopool.tile([128, 4 * (czb + 2) * N], f32, tag="lz0")
                lz0 = lz0t.rearrange("p (f z x) -> p f z x", f=4, z=czb + 2)
                for fi in range(4):
                    nc.sync.dma_start(
                        out=lz0[:, fi, :, :],
                        in_=bz[fi, tt0 - 1:tt0 + czb + 1, 0:128, :]
                        .transpose([1, 0, 2]))
                lz1t = iopool.tile([128, 4 * (hb + 2) * N], f32, tag="lz1")
                lz1 = lz1t.rearrange("p (f z x) -> p f z x", f=4, z=hb + 2)
                for half, pofs in ((0, 0), (1, 64)):
                    s0 = tt0 - 1 + half * hb
                    for fi in range(4):
                        nc.sync.dma_start(
                            out=lz1[pofs:pofs + 64, fi, :, :],
                            in_=bz[fi, s0:s0 + hb + 2, 128:192, :]
                            .transpose([1, 0, 2]))

                ly0t = iopool.tile([128, 4 * czb * N], f32, tag="ly0")
                ly0 = ly0t.rearrange("p (f z x) -> p f z x", f=4, z=czb)
                lx0t = iopool.tile([128, 4 * czb * N], f32, tag="lx0")
                lx0 = lx0t.rearrange("p (f z x) -> p f z x", f=4, z=czb)
                for buf, dst in ((by, ly0), (bx, lx0)):
                    for fi in range(4):
                        nc.sync.dma_start(
                            out=dst[:, fi, :, :],
                            in_=buf[fi, tt0:tt0 + czb, 0:128, :]
                            .transpose([1, 0, 2]))
                ly1t = scr.tile([64, 4 * czb * N], f32, tag="ly1")
                ly1 = ly1t.rearrange("p (f z x) -> p f z x", f=4, z=czb)
                for fi in range(4):
                    nc.sync.dma_start(
                        out=ly1[:, fi, :, :],
                        in_=by[fi, tt0:tt0 + czb, 128:192, :]
                        .transpose([1, 0, 2]))
                lx1t = scr.tile([128, 4 * hb * N], f32, tag="lx1")
                lx1 = lx1t.rearrange("p (f z x) -> p f z x", f=4, z=hb)
                for half, pofs in ((0, 0), (1, 64)):
                    s0 = tt0 + half * hb
                    for fi in range(4):
                        nc.sync.dma_start(
                            out=lx1[pofs:pofs + 64, fi, :, :],
                            in_=bx[fi, s0:s0 + hb, 128:192, :]
                            .transpose([1, 0, 2]))

                d0t = iopool.tile([128, 4 * czb * N], f32, tag="db0")
                dy0 = d0t.rearrange("p (f z x) -> p f z x", f=4, z=czb)
                d1t = iopool.tile([128, 4 * hb * N], f32, tag="db1")
                dy1 = d1t.rearrange("p (f z x) -> p f z x", f=4, z=hb)
                pe_dy(scr, ly0, ly1, dy0, dy1, czb)

                for (lzv, lxv, dyv, zc, half_mode) in (
                        (lz0, lx0, dy0, czb, False),
                        (lz1, lx1, dy1, hb, True)):
                    suf = "B" if half_mode else "A"
                    mt_ = scr.tile([128, 4 * zc * N], f32, tag="mb")
                    mv = mt_.rearrange("p (f z x) -> p f z x", f=4, z=zc)
                    nc.vector.tensor_sub(mv[:, :, :, :],
                                         lzv[:, :, 2:2 + zc, :],
                                         lzv[:, :, 0:zc, :])
                    xt_ = scr.tile([128, 4 * zc * N], f32, tag="xb")
                    xv = xt_.rearrange("p (f z x) -> p f z x", f=4, z=zc)
                    nc.vector.tensor_sub(xv[:, :, :, 1:191],
                                         lxv[:, :, :, 2:192],
                                         lxv[:, :, :, 0:190])
                    nc.vector.tensor_sub(xv[:, :, :, 0:192:191],
                                         lxv[:, :, :, 1::-1],
                                         lxv[:, :, :, 191:189:-1])
                    st_ = scr.tile([128, 4 * zc * N], f32, tag="ly1")
                    sv = st_.rearrange("p (f z x) -> p f z x", f=4, z=zc)
                    nc.vector.tensor_add(sv[:, :, :, :], mv[:, :, :, :],
                                         xv[:, :, :, :])
                    ot = scr.tile([128, 4 * zc * N], f32, tag="xb2")
                    ov = ot.rearrange("p (f z x) -> p f z x", f=4, z=zc)
                    nc.vector.tensor_add(ov[:, :, :, :], sv[:, :, :, :],
                                         dyv[:, :, :, :])
                    if not half_mode:
                        for fi in range(4):
                            nc.sync.dma_start(
                                out=out_d.ap()[fi, z:z + zc, 0:128, :]
                                .transpose([1, 0, 2]),
                                in_=ov[:, fi, :, :])
                    else:
                        for half, pofs in ((0, 0), (1, 64)):
                            s0 = z + half * hb
                            for fi in range(4):
                                nc.sync.dma_start(
                                    out=out_d.ap()[fi, s0:s0 + hb,
                                                   128:192, :]
                                    .transpose([1, 0, 2]),
                                    in_=ov[pofs:pofs + 64, fi, :, :])
                z += czb

            scrb_cm.__exit__(None, None, None)
            pass_b.__exit__(None, None, None)

    nc.compile()
    return nc


_NC_CACHE = None


def _get_nc():
    global _NC_CACHE
    if _NC_CACHE is None:
        _NC_CACHE = build_program()
    return _NC_CACHE


def make_dyt() -> np.ndarray:
    dm = np.zeros((N, N), dtype=np.float32)
    for m in range(N):
        dm[m, (m + 1) % N] = 1.0
        dm[m, (m - 1) % N] = -1.0
    return np.ascontiguousarray(dm.T)


def shard_inputs(u, T, nz=24, ncores=NCORES):
    dyt = make_dyt()
    in_maps = []
    for k in range(ncores):
        idx = np.arange(nz * k - 2, nz * k + nz + 2) % N
        in_maps.append({
            "u": np.ascontiguousarray(u[:, idx, :, :]),
            "T": np.ascontiguousarray(T[idx, :, :]),
            "dyt": dyt,
        })
    return in_maps


def kernel(u: np.ndarray, T: np.ndarray) -> np.ndarray:
    from concourse.bass_utils import run_bass_kernel_spmd

    u = np.asarray(u, dtype=np.float32)
    T = np.asarray(T, dtype=np.float32)
    nc = _get_nc()
    nz = N // NCORES
    in_maps = shard_inputs(u, T, nz=nz)
    res = run_bass_kernel_spmd(nc, in_maps, list(range(NCORES)))

    out = np.zeros((5, N, N, N), dtype=np.float32)
    for k in range(NCORES):
        out[1:5, nz * k:nz * k + nz, :, :] = res.results[k]["out"]
    return out

